# revision 12
# baseline (speedup 1.0000x reference)
"""Llama attention layer (B=2, S=2048, H=4096, 32 heads, fp32 io) on 8 trn2 cores.

Sharding: tensor-parallel over heads. Each core owns 4 heads: W_qkv column
shard [4096, 3*512] (bf16), W_o row shard [512, 4096] (bf16). Each core
computes qkv proj + RoPE + causal attention for its heads + its o_proj
partial; the host sums the 8 fp32 partials (the "all-reduce") and
untransposes the output (kernel emits o_partial^T).

v2 vs baseline (tensor-engine work reduction + stall elimination):
  - softmax row-sums no longer use 128*1*512 matmuls (94us of PE time);
    P tiles are accumulated on Vector and partition-summed on GpSimd.
  - diagonal causal blocks stream only the unmasked column range in the
    scores and PV matmuls (exp trimmed to match).
  - phase 3 is weight-stationary: out^T[o,t] = Wo_chunk^T @ attn^T tiles,
    so only a [128,4,128] Wo tile is resident at a time.
  - emission-level scheduling: phase-1 / phase-3 work is emitted in small
    units between phase-2 groups (lag-1 pipelined) so the PE never idles
    on the exp-activation chain and stays at full p-state.
  - hT / W_qkv / W_o are pre-arranged on host so every DMA line is one
    contiguous 1-8KB descriptor per partition.
"""

import numpy as np
import ml_dtypes

import concourse.bass as bass
import concourse.tile as tile
from concourse import bacc, mybir
from concourse.bass_isa import ReduceOp
from concourse.bass_utils import run_bass_kernel_spmd

# ---- problem constants (hardcoded per contract) ----
HIDDEN = 4096
NH = 32
D = 128
B = 2
S = 2048
TOK = B * S            # 4096 tokens
N_CORES = 8
HC = NH // N_CORES     # 4 heads per core
FH = HC * D            # 512 features per core for each of q/k/v
SCALING = D ** -0.5
ROPE_BASE = 10000.0

BF16 = mybir.dt.bfloat16
F32 = mybir.dt.float32

TBLK = 512             # tokens per phase-1 block
NTB = TOK // TBLK      # 8
QT = 512               # q columns per phase-2 tile
NQT = S // QT          # 4
NKC = S // 128         # 16 k chunks per sequence
NOB = HIDDEN // 128    # 32 output-column chunks in phase 3
EXP = mybir.ActivationFunctionType.Exp


class Filler:
    """Pulls emission units (generators yielding ~tensor-ns) on demand."""

    def __init__(self, gens):
        self.gens = list(gens)

    def pull(self, ns):
        while ns > 0 and self.gens:
            try:
                ns -= next(self.gens[0])
            except StopIteration:
                self.gens.pop(0)

    def drain(self):
        self.pull(float("inf"))


def build_nc():
    nc = bacc.Bacc("TRN2", target_bir_lowering=False, debug=False,
                   num_devices=N_CORES)
    hT = nc.dram_tensor("hT", [NTB, 4, 128, 8 * TBLK], BF16, kind="ExternalInput").ap()
    w = nc.dram_tensor("w", [3, 4, 128, 8 * FH], BF16, kind="ExternalInput").ap()
    wo = nc.dram_tensor("wo", [NOB, 128, HC, 128], BF16, kind="ExternalInput").ap()
    csn = nc.dram_tensor("csn", [TOK, 2, FH], BF16, kind="ExternalInput").ap()
    msk = nc.dram_tensor("msk", [128, 2, 2, QT], BF16, kind="ExternalInput").ap()
    outp = nc.dram_tensor("outp", [HIDDEN, TOK], F32, kind="ExternalOutput").ap()

    with tile.TileContext(nc) as tc:
        from contextlib import ExitStack
        with ExitStack() as ctx:
            # PSUM: ps 2 banks + ps2 4 banks + pv 2 banks = 8
            psp = ctx.enter_context(tc.tile_pool(name="ps", bufs=2, space="PSUM"))
            ps2p = ctx.enter_context(tc.tile_pool(name="ps2", bufs=2, space="PSUM"))
            pvp = ctx.enter_context(tc.tile_pool(name="pv", bufs=2, space="PSUM"))
            gsb = ctx.enter_context(tc.tile_pool(name="gsb", bufs=1))
            pairp = ctx.enter_context(tc.tile_pool(name="pair", bufs=6))
            ptp = ctx.enter_context(tc.tile_pool(name="pt", bufs=6))
            accp = ctx.enter_context(tc.tile_pool(name="acc", bufs=2))
            lnvp = ctx.enter_context(tc.tile_pool(name="lnv", bufs=2))
            aup = ctx.enter_context(tc.tile_pool(name="au", bufs=4))
            dscr = ctx.enter_context(tc.tile_pool(name="dscr", bufs=1, space="DRAM"))

            q_nat = dscr.tile([TOK, FH], BF16, tag="qs", name="q_nat")
            k_nat = dscr.tile([TOK, FH], BF16, tag="ks", name="k_nat")
            v_nat = dscr.tile([TOK, FH], BF16, tag="vs", name="v_nat")
            scr = [q_nat, k_nat, v_nat]

            attn_s = [gsb.tile([128, HC, S], BF16, tag="attn0", name="attn0"),
                      gsb.tile([128, HC, S], BF16, tag="attn1", name="attn1")]
            mskt = gsb.tile([128, 2, 2, QT], BF16, tag="msk", name="mskt")
            nc.sync.dma_start(out=mskt, in_=msk)

            # ---------------- phase 2: one (batch, head) pair ----------------
            def prefetch_pair(b, hh):
                rows = slice(b * S, (b + 1) * S)
                cols = slice(hh * D, (hh + 1) * D)
                qTp = pairp.tile([128, S], BF16, tag="pair", name="qTp")
                nc.sync.dma_start_transpose(out=qTp, in_=q_nat[rows, cols])
                kTp = pairp.tile([128, S], BF16, tag="pair", name="kTp")
                nc.sync.dma_start_transpose(out=kTp, in_=k_nat[rows, cols])
                vp = pairp.tile([128, NKC, D], BF16, tag="pair", name="vp")
                nc.sync.dma_start(
                    out=vp,
                    in_=v_nat[rows, cols].rearrange("(kc p) d -> p kc d", p=128))
                return qTp, kTp, vp

            pending_norm = []

            def flush_norm(keep=1):
                # deferred (by one qt) reciprocal+normalize: by flush time the
                # gpsimd all_reduce is long done, so Vector never blocks on it
                while len(pending_norm) > keep:
                    pending_norm.pop(0)()

            def run_pair(b, hh, tiles, filler):
                qTp, kTp, vp = tiles
                for qt in range(NQT):
                    flush_norm(keep=1)
                    nkc = 4 * (qt + 1)
                    pv = pvp.tile([128, QT], F32, tag="pv", name="pv")
                    acc = accp.tile([128, QT], F32, tag="acc", name="acc")
                    prev_pv = None
                    for g in range(nkc // 2):
                        diag = g >= 2 * qt
                        rs = [max(0, 128 * (2 * g + s2 - 4 * qt)) for s2 in (0, 1)]
                        sc = ps2p.tile([128, 2, QT], F32, tag="ps2", name="sc")
                        for s2 in range(2):
                            kc = 2 * g + s2
                            nc.tensor.matmul(
                                sc[:, s2, rs[s2]:],
                                lhsT=kTp[:, kc * 128:(kc + 1) * 128],
                                rhs=qTp[:, qt * QT + rs[s2]:(qt + 1) * QT],
                                start=True, stop=True)
                        pt2 = ptp.tile([128, 2, QT], BF16, tag="pt", name="pt2")
                        if diag:
                            for s2 in range(2):
                                r = rs[s2]
                                nc.scalar.activation(
                                    out=pt2[:, s2, r:], in_=sc[:, s2, r:],
                                    func=EXP, scale=SCALING)
                            psel = ptp.tile([128, 2, QT], BF16, tag="pt", name="ptm")
                            for s2 in range(2):
                                r = rs[s2]
                                nc.vector.tensor_mul(
                                    psel[:, s2, r:], pt2[:, s2, r:],
                                    mskt[:, g - 2 * qt, s2, r:])
                        else:
                            nc.scalar.activation(out=pt2, in_=sc, func=EXP,
                                                 scale=SCALING)
                            psel = pt2
                        # softmax denominator accumulation (Vector, fp32)
                        if g == 0:
                            if diag:  # qt == 0: rs == [0, 128]
                                nc.vector.tensor_copy(out=acc, in_=psel[:, 0, :])
                                nc.vector.tensor_add(
                                    acc[:, 128:], acc[:, 128:], psel[:, 1, 128:])
                            else:
                                nc.vector.tensor_add(acc, psel[:, 0, :], psel[:, 1, :])
                        else:
                            for s2 in range(2):
                                r = rs[s2]
                                nc.vector.tensor_add(
                                    acc[:, r:], acc[:, r:], psel[:, s2, r:])
                        filler.pull(900)
                        if prev_pv is not None:
                            prev_pv()

                        def mk_pv(psel_, g_, rs_):
                            def emit():
                                for s2 in range(2):
                                    kc = 2 * g_ + s2
                                    nc.tensor.matmul(
                                        pv[:, rs_[s2]:], lhsT=vp[:, kc, :],
                                        rhs=psel_[:, s2, rs_[s2]:],
                                        start=(kc == 0), stop=(kc == nkc - 1))
                            return emit
                        prev_pv = mk_pv(psel, g, rs)
                    filler.pull(400)
                    prev_pv()
                    # stage unnormalized attn to SBUF so the pv psum slot is
                    # released without waiting on the l-reduction chain
                    attn_u = aup.tile([128, QT], BF16, tag="au", name="attn_u")
                    nc.vector.tensor_copy(out=attn_u, in_=pv)
                    nc.gpsimd.partition_all_reduce(acc, acc, 128, ReduceOp.add)

                    def mk_norm(acc_, attn_u_, b_, hh_, qt_):
                        def emit():
                            linv = lnvp.tile([128, QT], F32, tag="lnv", name="linv")
                            nc.vector.reciprocal_approx_fast(out=linv, in_=acc_)
                            nc.vector.tensor_mul(
                                attn_s[b_][:, hh_, qt_ * QT:(qt_ + 1) * QT],
                                attn_u_, linv)
                        return emit
                    pending_norm.append(mk_norm(acc, attn_u, b, hh, qt))

            # ---------------- phase 1: qkv projections + rope ----------------
            def p1_stream(blocks, pools):
                hp, wp, csp, rtp, stp = pools
                for T in blocks:
                    hblk = []
                    for i in range(4):
                        t_ = hp.tile([128, 8, TBLK], BF16, tag="hblk", name="hblk")
                        src = hT[T, i].rearrange("p (kc t) -> p kc t", kc=8)
                        if T == blocks[0] and i == 0:
                            for kq in range(4):  # finer first-load: compute starts sooner
                                nc.sync.dma_start(
                                    out=t_[:, 2 * kq:2 * kq + 2, :],
                                    in_=src[:, 2 * kq:2 * kq + 2, :])
                        else:
                            nc.sync.dma_start(out=t_, in_=src)
                        hblk.append(t_)
                    csts = []
                    for tt in range(4):
                        cst = csp.tile([128, 2, HC, D], BF16, tag="cs", name="cst")
                        r0 = T * TBLK + tt * 128
                        nc.sync.dma_start(
                            out=cst,
                            in_=csn[r0:r0 + 128].rearrange("p c (h d) -> p c h d", h=HC))
                        csts.append(cst)
                    yield 0
                    for j3 in range(3):
                        wch = []
                        for i in range(4):
                            t_ = wp.tile([128, 8, FH], BF16, tag="wch", name="wch")
                            src = w[j3, i].rearrange("p (kc f) -> p kc f", kc=8)
                            if T == blocks[0] and j3 == 0 and i == 0:
                                for kq in range(4):
                                    nc.sync.dma_start(
                                        out=t_[:, 2 * kq:2 * kq + 2, :],
                                        in_=src[:, 2 * kq:2 * kq + 2, :])
                            else:
                                nc.sync.dma_start(out=t_, in_=src)
                            wch.append(t_)
                        yield 0
                        for tt in range(4):
                            ps = psp.tile([128, HC, D], F32, tag="ps", name="ps")
                            for half in range(8):
                                i = half // 2
                                for kc in range(4 * (half % 2), 4 * (half % 2) + 4):
                                    nc.tensor.matmul(
                                        ps,
                                        lhsT=hblk[i][:, kc, tt * 128:(tt + 1) * 128],
                                        rhs=wch[i][:, kc, :],
                                        start=(half == 0 and kc == 0),
                                        stop=(half == 7 and kc == 7),
                                    )
                                yield 852
                            st = stp.tile([128, HC, D], BF16, tag="stage", name="st")
                            if j3 < 2:
                                cst = csts[tt]
                                half_d = D // 2
                                tr = rtp.tile([128, HC, D], F32, tag="rtmp", name="tr")
                                tcos = rtp.tile([128, HC, D], F32, tag="rtmp", name="tcos")
                                nc.vector.tensor_mul(
                                    tr[:, :, 0:half_d], ps[:, :, half_d:D],
                                    cst[:, 1, :, 0:half_d])
                                nc.vector.tensor_mul(
                                    tr[:, :, half_d:D], ps[:, :, 0:half_d],
                                    cst[:, 1, :, half_d:D])
                                nc.vector.tensor_mul(tcos, ps, cst[:, 0])
                                nc.vector.tensor_add(st, tr, tcos)
                            else:
                                nc.vector.tensor_copy(out=st, in_=ps)
                            r0 = T * TBLK + tt * 128
                            nc.sync.dma_start(out=scr[j3][r0:r0 + 128, :], in_=st)
                            yield 0

            # ---------------- phase 3: o_proj partial (transposed out) -------
            def p3_stream(tbs, pools):
                # ob processed in pairs: alternating the two independent unit
                # chains covers each chain's psum-slot WAR with the other's
                # matmuls, so the PE never waits on a drain copy.
                wop, ostp = pools
                for obp in range(0, NOB, 2):
                    wots = []
                    for ob in (obp, obp + 1):
                        wot = wop.tile([128, HC, 128], BF16, tag="wo", name="wot")
                        nc.sync.dma_start(out=wot, in_=wo[ob])
                        wots.append(wot)
                    yield 0
                    for n, tb in enumerate(tbs):
                        for m, ob in enumerate((obp, obp + 1)):
                            pso = psp.tile([128, TBLK], F32, tag="ps", name="pso")
                            for kc in range(HC):
                                nc.tensor.matmul(
                                    pso, lhsT=wots[m][:, kc, :],
                                    rhs=attn_s[tb // 4][:, kc,
                                                        (tb % 4) * TBLK:(tb % 4 + 1) * TBLK],
                                    start=(kc == 0), stop=(kc == HC - 1))
                            yield 852
                            ot = ostp.tile([128, TBLK], F32, tag="ost", name="ot")
                            if (2 * n + m) % 2 == 0:
                                nc.vector.tensor_copy(out=ot, in_=pso)
                            else:
                                nc.scalar.copy(out=ot, in_=pso)
                            nc.sync.dma_start(
                                out=outp[ob * 128:(ob + 1) * 128,
                                         tb * TBLK:(tb + 1) * TBLK], in_=ot)
                            yield 0

            # ---------------- schedule -------------------------------------
            with ExitStack() as p1ctx:
                p1pools = (
                    p1ctx.enter_context(tc.tile_pool(name="hblk", bufs=6)),
                    p1ctx.enter_context(tc.tile_pool(name="wch", bufs=6)),
                    p1ctx.enter_context(tc.tile_pool(name="cs", bufs=5)),
                    p1ctx.enter_context(tc.tile_pool(name="rtmp", bufs=3)),
                    p1ctx.enter_context(tc.tile_pool(name="stage", bufs=4)),
                )
                Filler([p1_stream(range(4), p1pools)]).drain()
                fb = Filler([p1_stream(range(4, NTB), p1pools)])
                tiles = prefetch_pair(0, 0)
                fb.pull(12000)
                for hh in range(HC):
                    nxt = prefetch_pair(0, hh + 1) if hh < HC - 1 else None
                    run_pair(0, hh, tiles, fb)
                    tiles = nxt
                flush_norm(keep=0)
                fb.drain()

            with ExitStack() as p3ctx:
                p3pools = (
                    p3ctx.enter_context(tc.tile_pool(name="wop", bufs=3)),
                    p3ctx.enter_context(tc.tile_pool(name="ost", bufs=4)),
                )
                tiles = prefetch_pair(1, 0)
                fc = Filler([p3_stream(range(4), p3pools)])
                fc.pull(12000)
                for hh in range(HC):
                    nxt = prefetch_pair(1, hh + 1) if hh < HC - 1 else None
                    run_pair(1, hh, tiles, fc)
                    tiles = nxt
                flush_norm(keep=0)
                fc.drain()
                Filler([p3_stream(range(4, NTB), p3pools)]).drain()

    nc.compile()
    return nc


_NC_CACHE = {}


def get_nc():
    if "nc" not in _NC_CACHE:
        _NC_CACHE["nc"] = build_nc()
    return _NC_CACHE["nc"]


def prep_in_maps(positions, hidden_states, W_qkv, W_o):
    """Host-side sharding + layout prep. Returns per-core input maps."""
    bf16 = ml_dtypes.bfloat16
    hid = np.asarray(hidden_states, np.float32).reshape(TOK, HIDDEN)
    # hT[T, i, p, kc, t] = hid[T*512+t, i*1024+kc*128+p]
    hT = np.ascontiguousarray(
        hid.reshape(NTB, TBLK, 4, 8, 128).transpose(0, 2, 4, 3, 1)
    ).reshape(NTB, 4, 128, 8 * TBLK).astype(bf16)

    pos = np.asarray(positions).reshape(TOK).astype(np.float32)
    half = D // 2
    inv = ROPE_BASE ** (-np.arange(half, dtype=np.float32) / half)
    ang = pos[:, None] * inv[None, :]                      # [TOK, 64]
    cos = np.cos(ang)
    sin = np.sin(ang)
    cos128 = np.concatenate([cos, cos], axis=1)            # [TOK, 128]
    sin128 = np.concatenate([-sin, sin], axis=1)
    csn = np.empty((TOK, 2, FH), np.float32)
    csn[:, 0, :] = np.tile(cos128, HC)
    csn[:, 1, :] = np.tile(sin128, HC)
    csn = csn.astype(bf16)

    kk = np.arange(128)[:, None]
    qq = np.arange(QT)[None, :]
    msk = np.stack([(qq >= kk + o * 128) for o in range(4)], axis=1)
    msk = msk.reshape(128, 2, 2, QT).astype(bf16)           # [128, 2, 2, 512]

    Wq = np.asarray(W_qkv, np.float32)
    Wo = np.asarray(W_o, np.float32)
    in_maps = []
    for c in range(N_CORES):
        wc = np.concatenate(
            [Wq[:, q0 * HIDDEN + c * FH: q0 * HIDDEN + (c + 1) * FH]
             for q0 in range(3)], axis=1)                   # [4096, 1536]
        # w[j3, i, p, kc, f] = wc[i*1024+kc*128+p, j3*512+f]
        wcp = np.ascontiguousarray(
            wc.reshape(4, 8, 128, 3, FH).transpose(3, 0, 2, 1, 4)
        ).reshape(3, 4, 128, 8 * FH).astype(bf16)
        woc = Wo[c * FH:(c + 1) * FH, :]                    # [512, 4096]
        # wo[ob, p, kc, o] = woc[kc*128+p, ob*128+o]
        wop = np.ascontiguousarray(
            woc.reshape(HC, 128, NOB, 128).transpose(2, 1, 0, 3)
        ).astype(bf16)
        in_maps.append({"hT": hT, "w": wcp, "wo": wop, "csn": csn, "msk": msk})
    return in_maps


def combine_outputs(outps):
    """Sum per-core o_partial^T [HIDDEN, TOK] and untranspose."""
    out = outps[0].astype(np.float64)
    for o in outps[1:]:
        out += o
    return np.ascontiguousarray(out.T).astype(np.float32).reshape(B, S, HIDDEN)


def kernel(positions, hidden_states, W_qkv, W_o):
    nc = get_nc()
    in_maps = prep_in_maps(positions, hidden_states, W_qkv, W_o)
    res = run_bass_kernel_spmd(nc, in_maps, list(range(N_CORES)))
    return combine_outputs([res.results[c]["outp"] for c in range(N_CORES)])
